# revision 9
# baseline (speedup 1.0000x reference)
"""Masked attention (out, p_attn) Trainium2 Bass kernel, SPMD over 8 NeuronCores.

Full inputs:  query/key/value [8, 16, 1024, 64] f32, mask [8, 16, 1024] int32.
Sharding:     core i <- batch b=i (16 heads per core, full attention per head).
Reference semantics (including the scale-by-sqrt(N)=32 "bug"):
    scores = (q @ k^T) / 32 ; scores[mask==0] = -1e9
    p = softmax(scores, axis=-1) ; out = p @ v ; return (out, p)

Per-head pipeline on one core:
    - load Q,K,V f32, convert to bf16
    - PE-transpose Q,K into [65, seq] tiles: row 64 of Q^T is ones, row 64 of
      K^T is (mask-1)*3.2e10 so that the additive -1e9 mask (post-scale) fuses
      into the QK^T matmul as a 65th contraction row.
    - MM1: S[mc] = Q'^T[:,mc]^T @ K'^T  -> PSUM [128, 1024] f32
    - ACT exp with scale=1/32 fused, accum_out = row sums (softmax denom)
    - PE-transpose E -> E^T (needed so MM2 contracts over n on partitions)
    - MM2: out[mc] += E^T[nc, mc]^T @ V[nc]  (accumulate over n chunks)
    - normalize: p = E * (1/denom) (GPSIMD), out = psum * (1/denom) (DVE)
    - batched HWDGE DMA out (p_attn 4MB/head, out 256KB/head)
"""

import os
import sys
from contextlib import ExitStack

import numpy as np

for _p in ("/opt/trn_rl_repo", "/root/.axon_site/_ro/trn_rl_repo"):
    if os.path.isdir(_p) and _p not in sys.path:
        sys.path.insert(0, _p)

import concourse.bacc as bacc
import concourse.bass as bass
import concourse.tile as tile
from concourse import mybir
from concourse.bass_utils import run_bass_kernel_spmd
from concourse.masks import make_identity

F32 = mybir.dt.float32
BF16 = mybir.dt.bfloat16
I32 = mybir.dt.int32

B, H, M, N, D = 8, 16, 1024, 1024, 64
P = 128
MC = M // P  # 8 m-chunks per head
NCH = N // P  # 8 n-chunks per head
SCALE = float(1.0 / np.sqrt(np.float32(N)))  # 1/32, applied inside exp
MASK_BIAS = -1e9 / SCALE  # pre-scale additive bias for masked columns

N_CORES = 8


def build_bass(n_heads=H, do_compile=True):
    nc = bacc.Bacc()
    q_ext = nc.declare_dram_parameter("query", [n_heads, M, D], F32, isOutput=False)
    k_ext = nc.declare_dram_parameter("key", [n_heads, N, D], F32, isOutput=False)
    v_ext = nc.declare_dram_parameter("value", [n_heads, N, D], F32, isOutput=False)
    m_ext = nc.declare_dram_parameter("mask", [n_heads, N], I32, isOutput=False)
    out_ext = nc.declare_dram_parameter("out", [n_heads, M, D], F32, isOutput=True)
    p_ext = nc.declare_dram_parameter("p_attn", [n_heads, M, N], F32, isOutput=True)

    with ExitStack() as ctx:
        tc = ctx.enter_context(tile.TileContext(nc))
        const = ctx.enter_context(tc.tile_pool(name="const", bufs=1))
        ident = const.tile([P, P], BF16)
        make_identity(nc, ident)

        # mask -> (mask-1)*3.2e10 for all heads at once, bf16 [n_heads, N]
        mask_i32 = const.tile([n_heads, N], I32)
        nc.sync.dma_start(out=mask_i32, in_=m_ext[:, :])
        maskbias = const.tile([n_heads, N], BF16)
        nc.vector.tensor_scalar(
            out=maskbias,
            in0=mask_i32,
            scalar1=-1.0,
            scalar2=-MASK_BIAS,  # (mask-1) * 3.2e10 -> masked cols get -3.2e10
            op0=mybir.AluOpType.add,
            op1=mybir.AluOpType.mult,
        )

        loads = ctx.enter_context(tc.tile_pool(name="loads", bufs=2))
        bfp = ctx.enter_context(tc.tile_pool(name="bfp", bufs=2))
        qkt = ctx.enter_context(tc.tile_pool(name="qkt", bufs=2))
        epool = ctx.enter_context(tc.tile_pool(name="epool", bufs=2))
        etpool = ctx.enter_context(tc.tile_pool(name="etpool", bufs=2))
        ppool = ctx.enter_context(tc.tile_pool(name="ppool", bufs=2))
        opool = ctx.enter_context(tc.tile_pool(name="opool", bufs=2))
        small = ctx.enter_context(tc.tile_pool(name="small", bufs=4))

        psum_s = ctx.enter_context(tc.tile_pool(name="psum_s", bufs=2, space="PSUM"))
        psum_t = ctx.enter_context(tc.tile_pool(name="psum_t", bufs=2, space="PSUM"))
        psum_o = ctx.enter_context(tc.tile_pool(name="psum_o", bufs=2, space="PSUM"))

        for h in range(n_heads):
            q_dram = q_ext[h].rearrange("(c p) d -> p c d", p=P)  # [128, 8, 64]
            k_dram = k_ext[h].rearrange("(c p) d -> p c d", p=P)
            v_dram = v_ext[h].rearrange("(c p) d -> p c d", p=P)

            q_f32 = loads.tile([P, MC, D], F32, tag="qf")
            k_f32 = loads.tile([P, NCH, D], F32, tag="kf")
            v_f32 = loads.tile([P, NCH, D], F32, tag="vf")
            nc.sync.dma_start(out=q_f32, in_=q_dram)
            nc.sync.dma_start(out=k_f32, in_=k_dram)
            nc.sync.dma_start(out=v_f32, in_=v_dram)

            q_bf = bfp.tile([P, MC, D], BF16, tag="qb")
            k_bf = bfp.tile([P, NCH, D], BF16, tag="kb")
            v_bf = bfp.tile([P, NCH, D], BF16, tag="vb")
            nc.vector.tensor_copy(out=q_bf, in_=q_f32)
            nc.vector.tensor_copy(out=k_bf, in_=k_f32)
            nc.vector.tensor_copy(out=v_bf, in_=v_f32)

            # Build Q'^T [65, M] and K'^T [65, N] in SBUF via PE transposes.
            qT = qkt.tile([P // 2 + 1, M], BF16, tag="qT")  # 65 partitions
            kT = qkt.tile([P // 2 + 1, N], BF16, tag="kT")
            for (src, dstT, ext_cols) in ((q_bf, qT, MC), (k_bf, kT, NCH)):
                t_ps = psum_t.tile([P, max(M, N)], BF16, tag="tps")
                for c in range(ext_cols):
                    nc.tensor.transpose(
                        t_ps[:D, c * P : (c + 1) * P], src[:, c, :], ident
                    )
                nc.vector.tensor_copy(out=dstT[:D, :], in_=t_ps[:D, :])
            nc.vector.memset(qT[D : D + 1, :], 1.0)
            # row 64 of K'^T = per-column additive mask bias (cross-partition
            # move done by a small SBUF->SBUF DMA)
            nc.sync.dma_start(out=kT[D : D + 1, :], in_=maskbias[h : h + 1, :])

            # MM1 + exp + denominators
            e_sb = epool.tile([P, MC, N], BF16, tag="e")
            denom = small.tile([P, MC], F32, tag="den")
            for mc in range(MC):
                s_ps = psum_s.tile([P, N], F32, tag="s")
                for half in range(N // 512):
                    nc.tensor.matmul(
                        s_ps[:, half * 512 : (half + 1) * 512],
                        qT[:, mc * P : (mc + 1) * P],
                        kT[:, half * 512 : (half + 1) * 512],
                        start=True,
                        stop=True,
                    )
                nc.scalar.activation(
                    out=e_sb[:, mc, :],
                    in_=s_ps,
                    func=mybir.ActivationFunctionType.Exp,
                    scale=SCALE,
                    accum_out=denom[:, mc : mc + 1],
                )

            recip = small.tile([P, MC], F32, tag="rec")
            nc.vector.reciprocal(out=recip, in_=denom)

            # Transpose E -> E^T
            et_sb = etpool.tile([P, NCH, M], BF16, tag="et")
            for ncx in range(NCH):
                t_ps = psum_t.tile([P, max(M, N)], BF16, tag="tps")
                for mc in range(MC):
                    nc.tensor.transpose(
                        t_ps[:, mc * P : (mc + 1) * P],
                        e_sb[:, mc, ncx * P : (ncx + 1) * P],
                        ident,
                    )
                nc.vector.tensor_copy(out=et_sb[:, ncx, :], in_=t_ps[:, :M])

            # MM2 + output normalize
            o_sb = opool.tile([P, MC, D], F32, tag="o")
            for mc in range(MC):
                o_ps = psum_o.tile([P, D], F32, tag="ops")
                for ncx in range(NCH):
                    nc.tensor.matmul(
                        o_ps,
                        et_sb[:, ncx, mc * P : (mc + 1) * P],
                        v_bf[:, ncx, :],
                        start=(ncx == 0),
                        stop=(ncx == NCH - 1),
                    )
                nc.vector.tensor_scalar_mul(
                    out=o_sb[:, mc, :], in0=o_ps, scalar1=recip[:, mc : mc + 1]
                )

            # p_attn normalize on GPSIMD (bf16 -> f32)
            p_sb = ppool.tile([P, MC, N], F32, tag="p")
            for mc in range(MC):
                nc.gpsimd.tensor_scalar_mul(
                    out=p_sb[:, mc, :], in0=e_sb[:, mc, :], scalar1=recip[:, mc : mc + 1]
                )

            out_dram = out_ext[h].rearrange("(c p) d -> p c d", p=P)
            p_dram = p_ext[h].rearrange("(c p) n -> p c n", p=P)
            nc.sync.dma_start(out=out_dram, in_=o_sb)
            nc.scalar.dma_start(out=p_dram, in_=p_sb)

    if do_compile:
        nc.compile()
    return nc


_CACHED_NC = None


def _get_nc():
    global _CACHED_NC
    if _CACHED_NC is None:
        _CACHED_NC = build_bass()
    return _CACHED_NC


def run_sharded(query, key, value, mask, trace=False, **kw):
    """Run the SPMD kernel; returns (results, BassKernelResults)."""
    query = np.ascontiguousarray(np.asarray(query, dtype=np.float32))
    key = np.ascontiguousarray(np.asarray(key, dtype=np.float32))
    value = np.ascontiguousarray(np.asarray(value, dtype=np.float32))
    mask = np.ascontiguousarray(np.asarray(mask, dtype=np.int32))
    assert query.shape == (B, H, M, D), query.shape

    nc = _get_nc()
    in_maps = [
        {
            "query": query[i],
            "key": key[i],
            "value": value[i],
            "mask": mask[i],
        }
        for i in range(N_CORES)
    ]
    res = run_bass_kernel_spmd(nc, in_maps, list(range(N_CORES)), trace=trace, **kw)
    results = res.results
    out = np.stack([results[i]["out"] for i in range(N_CORES)], axis=0)
    p_attn = np.stack([results[i]["p_attn"] for i in range(N_CORES)], axis=0)
    return (out, p_attn), res


def kernel(query, key, value, mask):
    (out, p_attn), _ = run_sharded(query, key, value, mask, trace=False)
    return out, p_attn


# revision 17
# speedup vs baseline: 1.0865x; 1.0865x over previous
"""Masked attention (out, p_attn) Trainium2 Bass kernel, SPMD over 8 NeuronCores.

Full inputs:  query/key/value [8, 16, 1024, 64] f32, mask [8, 16, 1024] int32.
Sharding:     core i <- batch b=i (16 heads per core, full attention per head).
Reference semantics (including the scale-by-sqrt(N)=32 "bug"):
    scores = (q @ k^T) / 32 ; scores[mask==0] = -1e9
    p = softmax(scores, axis=-1) ; out = p @ v ; return (out, p)

This axon environment is per-instruction / per-descriptor overhead dominated
(measured: ~2.1us per matmul call, ~6.5us per [128,1024] DVE op, ~0.6us per
DMA descriptor that writes ExternalOutput DRAM, while contiguous DMA and
internal-DRAM writes run near full bandwidth). Design choices:

- Host preprocessing inside kernel(): q/k are transposed, scaled-mask row
  appended ([65, 1024]: row 64 of q' = ones, row 64 of k' = (mask-1)*3.2e10
  so the additive -1e9 mask fuses into the QK matmul as a 65th contraction
  row), and cast to bf16. v is pre-cast to bf16. No device-side transposes
  or converts for the inputs.
- MM1 (16 calls/head): S[mc] = q'^T[:,mc]^T @ k'^T -> PSUM [128,1024] f32.
- exp on ACT (8 ops/head) with scale=1/32 fused and accum_out giving the
  softmax denominators for free.
- E^T for MM2 either via 64 per-block xbar DMA transposes (contiguous dests,
  the known-good pattern) or via a second set of 16 matmuls computing S^T
  plus 8 more exps (ET_MODE below).
- MM2 (16 calls/head): out^T[64,1024] += V[nc]-stationary @ E^T[nc]-moving.
- out^T -> bf16 -> 8 xbar transposes -> [128,8,64]; normalize by 1/denom.
- p = E * (1/denom) on DVE (8 ops/head).
- p/out staged to internal DRAM (strided writes there are cheap), then one
  contiguous DRAM->DRAM copy per head to the external outputs (16 large
  descriptors instead of 1024 small ones).
"""

import os
import sys
from contextlib import ExitStack

import numpy as np

for _p in ("/opt/trn_rl_repo", "/root/.axon_site/_ro/trn_rl_repo"):
    if os.path.isdir(_p) and _p not in sys.path:
        sys.path.insert(0, _p)

import ml_dtypes
import concourse.bacc as bacc
import concourse.bass as bass
import concourse.tile as tile
from concourse import mybir
from concourse.bass_utils import run_bass_kernel_spmd

F32 = mybir.dt.float32
BF16 = mybir.dt.bfloat16

B, H, M, N, D = 8, 16, 1024, 1024, 64
P = 128
MC = M // P  # 8 m-chunks per head
NCH = N // P  # 8 n-chunks per head
SCALE = float(1.0 / np.sqrt(np.float32(N)))  # 1/32, fused into exp
MASK_BIAS = -1e9 / SCALE  # pre-scale additive bias for masked columns

N_CORES = 8
ET_MODE = "xbar"  # "xbar" (64 block transposes) or "mm1b" (second matmul set)


def build_bass(n_heads=H, do_compile=True, reps=1, et_mode=None):
    et_mode = et_mode or ET_MODE
    nc = bacc.Bacc()
    q_ext = nc.declare_dram_parameter("q_aug", [n_heads, D + 1, M], BF16, False)
    k_ext = nc.declare_dram_parameter("k_aug", [n_heads, D + 1, N], BF16, False)
    v_ext = nc.declare_dram_parameter("v_bf", [n_heads, N, D], BF16, False)
    out_ext = nc.declare_dram_parameter("out", [n_heads, M, D], F32, True)
    p_ext = nc.declare_dram_parameter("p_attn", [n_heads, M, N], F32, True)

    with ExitStack() as ctx:
        tc = ctx.enter_context(tile.TileContext(nc))
        dramp = ctx.enter_context(tc.tile_pool(name="dramp", bufs=2, space="DRAM"))
        qkt = ctx.enter_context(tc.tile_pool(name="qkt", bufs=2))
        vpool = ctx.enter_context(tc.tile_pool(name="vpool", bufs=2))
        epool = ctx.enter_context(tc.tile_pool(name="epool", bufs=2))
        etpool = ctx.enter_context(tc.tile_pool(name="etpool", bufs=2))
        ppool = ctx.enter_context(tc.tile_pool(name="ppool", bufs=2))
        opool = ctx.enter_context(tc.tile_pool(name="opool", bufs=2))
        small = ctx.enter_context(tc.tile_pool(name="small", bufs=4))

        psum_s = ctx.enter_context(tc.tile_pool(name="psum_s", bufs=2, space="PSUM"))
        psum_o = ctx.enter_context(tc.tile_pool(name="psum_o", bufs=2, space="PSUM"))

        for h in list(range(n_heads)) * reps:
            qT = qkt.tile([D + 1, M], BF16, tag="qT")
            kT = qkt.tile([D + 1, N], BF16, tag="kT")
            v_bf = vpool.tile([P, NCH, D], BF16, tag="vb")
            nc.sync.dma_start(out=qT, in_=q_ext[h])
            nc.sync.dma_start(out=kT, in_=k_ext[h])
            nc.sync.dma_start(
                out=v_bf, in_=v_ext[h].rearrange("(c p) d -> p c d", p=P)
            )

            # MM1 + exp + denominators
            e_sb = epool.tile([P, MC, N], BF16, tag="e")
            denom = small.tile([P, MC], F32, tag="den")
            for mc in range(MC):
                s_ps = psum_s.tile([P, N], F32, tag="s")
                for half in range(N // 512):
                    nc.tensor.matmul(
                        s_ps[:, half * 512 : (half + 1) * 512],
                        qT[:, mc * P : (mc + 1) * P],
                        kT[:, half * 512 : (half + 1) * 512],
                        start=True,
                        stop=True,
                    )
                nc.scalar.activation(
                    out=e_sb[:, mc, :],
                    in_=s_ps,
                    func=mybir.ActivationFunctionType.Exp,
                    scale=SCALE,
                    accum_out=denom[:, mc : mc + 1],
                )

            recip = small.tile([P, MC], F32, tag="rec")
            nc.vector.reciprocal(out=recip, in_=denom)

            # E^T [n%128, ncx, mc, 128]
            et = etpool.tile([P, NCH, MC, P], BF16, tag="et")
            if et_mode == "xbar":
                for mc in range(MC):
                    for ncx in range(NCH):
                        nc.scalar.dma_start_transpose(
                            et[:, ncx, mc, :],
                            e_sb[:, mc, ncx * P : (ncx + 1) * P],
                        )
            else:  # mm1b: compute S^T directly, exp again
                for ncx in range(NCH):
                    st_ps = psum_s.tile([P, M], F32, tag="s")
                    for half in range(M // 512):
                        nc.tensor.matmul(
                            st_ps[:, half * 512 : (half + 1) * 512],
                            kT[:, ncx * P : (ncx + 1) * P],
                            qT[:, half * 512 : (half + 1) * 512],
                            start=True,
                            stop=True,
                        )
                    nc.scalar.activation(
                        out=et[:, ncx].rearrange("p a k -> p (a k)"),
                        in_=st_ps,
                        func=mybir.ActivationFunctionType.Exp,
                        scale=SCALE,
                    )

            # MM2: out^T [64, M] += V[ncx] stationary @ E^T[ncx] moving
            ot_ps = psum_o.tile([D, M], F32, tag="ot")
            for ncx in range(NCH):
                rhs = et[:, ncx].rearrange("p a k -> p (a k)")  # [128, 1024]
                for half in range(M // 512):
                    nc.tensor.matmul(
                        ot_ps[:, half * 512 : (half + 1) * 512],
                        v_bf[:, ncx, :],
                        rhs[:, half * 512 : (half + 1) * 512],
                        start=(ncx == 0),
                        stop=(ncx == NCH - 1),
                    )

            # out^T -> bf16 -> xbar per-slice transposes -> [128, 8, 64]
            ot_bf = opool.tile([D, M], BF16, tag="otbf")
            nc.vector.tensor_copy(out=ot_bf, in_=ot_ps)
            o_tr = opool.tile([P, MC, D], BF16, tag="otr")
            for mc in range(MC):
                nc.scalar.dma_start_transpose(
                    o_tr[:, mc, :], ot_bf[:, mc * P : (mc + 1) * P]
                )
            o_sb = opool.tile([P, MC, D], F32, tag="o")
            for mc in range(MC):
                nc.vector.tensor_scalar_mul(
                    out=o_sb[:, mc, :], in0=o_tr[:, mc, :],
                    scalar1=recip[:, mc : mc + 1],
                )

            # p = E * recip (DVE), staged f32
            p_sb = ppool.tile([P, MC, N], F32, tag="p")
            for mc in range(MC):
                nc.vector.tensor_scalar_mul(
                    out=p_sb[:, mc, :], in0=e_sb[:, mc, :],
                    scalar1=recip[:, mc : mc + 1],
                )

            # stores: strided to internal scratch (cheap), then contiguous
            # D2D per head to the external outputs (few big descriptors).
            o_scr = dramp.tile([M, D], F32, tag="oscr")
            p_scr = dramp.tile([M, N], F32, tag="pscr")
            nc.sync.dma_start(
                out=o_scr.rearrange("(c p) d -> p c d", p=P), in_=o_sb
            )
            nc.sync.dma_start(
                out=p_scr.rearrange("(c p) n -> p c n", p=P), in_=p_sb
            )
            nc.sync.dma_start(
                out=out_ext[h].rearrange("m d -> (m d)"),
                in_=o_scr.rearrange("m d -> (m d)"),
            )
            nc.sync.dma_start(
                out=p_ext[h].rearrange("m n -> (m n)"),
                in_=p_scr.rearrange("m n -> (m n)"),
            )

    if do_compile:
        nc.compile()
    return nc


def host_prep(query, key, value, mask):
    """Per-core input maps with host-side transpose/augment/cast."""
    query = np.asarray(query, dtype=np.float32)
    key = np.asarray(key, dtype=np.float32)
    value = np.asarray(value, dtype=np.float32)
    mask = np.asarray(mask)
    bh = query.shape[0]
    q_aug = np.empty((bh, H, D + 1, M), dtype=ml_dtypes.bfloat16)
    k_aug = np.empty((bh, H, D + 1, N), dtype=ml_dtypes.bfloat16)
    q_aug[:, :, :D, :] = query.transpose(0, 1, 3, 2)
    q_aug[:, :, D, :] = 1.0
    k_aug[:, :, :D, :] = key.transpose(0, 1, 3, 2)
    k_aug[:, :, D, :] = ((mask.astype(np.float32) - 1.0) * -MASK_BIAS).astype(
        ml_dtypes.bfloat16
    )
    v_bf = value.astype(ml_dtypes.bfloat16)
    return [
        {"q_aug": q_aug[i], "k_aug": k_aug[i], "v_bf": v_bf[i]}
        for i in range(bh)
    ]


_CACHED_NC = None


def _get_nc():
    global _CACHED_NC
    if _CACHED_NC is None:
        _CACHED_NC = build_bass()
    return _CACHED_NC


def run_sharded(query, key, value, mask, trace=False, **kw):
    """Run the SPMD kernel; returns ((out, p_attn), BassKernelResults)."""
    assert np.asarray(query).shape == (B, H, M, D)
    in_maps = host_prep(query, key, value, mask)
    nc = _get_nc()
    res = run_bass_kernel_spmd(nc, in_maps, list(range(N_CORES)), trace=trace, **kw)
    results = res.results
    out = np.stack([results[i]["out"] for i in range(N_CORES)], axis=0)
    p_attn = np.stack([results[i]["p_attn"] for i in range(N_CORES)], axis=0)
    return (out, p_attn), res


def kernel(query, key, value, mask):
    (out, p_attn), _ = run_sharded(query, key, value, mask, trace=False)
    return out, p_attn
